# revision 7
# baseline (speedup 1.0000x reference)
"""BiMamba layer on 8 TRN2 NeuronCores — v2.

Sharding: 8 cores = 4 (dir,batch) pairs x 2 halves of d_inner; host flips
the sequence for the backward direction, transposes to [channel, token]
layout, and sums the 4 partial outputs per batch + residual at the end.

v2 changes vs baseline (driven by HW microbenchmarks):
  - LN folded into in_proj: xs = x*rstd (DVE); the -mu*rstd and bias
    terms ride a K=2 rank-1 matmul accumulated into the in_proj PSUM.
    xT loaded once and kept resident (baseline streamed it twice).
  - in_proj in bf16 (was f32r).
  - depthwise conv on the PE as 4 shifted diag-matmuls (was DVE
    scalar_tensor_tensor at 1x); SiLU fused into the PSUM evacuation
    via the ACT Silu table (kills sigmoid+mult on DVE).
  - z-SiLU fused into the in_proj z evacuation (ACT Silu).
  - scan phase: j-pairs fused into [128, 2L] tiles: one exp + one scan
    per (n,pair); a poison column (dt=30) at the segment boundary kills
    the scan carry between the two channel tiles. B/C broadcasts ride
    two HWDGE queues (sync=B, scalar=C), double-buffered.
  - u*D folded into the scan PSUM via a diag(D) matmul.
GPSIMD left idle on purpose: its SBUF port is shared with the DVE and
concurrent use halves both engines' throughput (measured).
"""
import sys
sys.path.insert(0, '/opt/trn_rl_repo')
import numpy as np
import ml_dtypes
from contextlib import ExitStack

import concourse.bass as bass
import concourse.tile as tile
from concourse import bacc, mybir
from concourse.bass_utils import run_bass_kernel_spmd

AF = mybir.ActivationFunctionType
OP = mybir.AluOpType
F32, BF16, F32R = mybir.dt.float32, mybir.dt.bfloat16, mybir.dt.float32r
BF = ml_dtypes.bfloat16

DIM, DSTATE, DCONV, DINNER, DTRANK, B, L = 512, 16, 4, 1024, 32, 2, 2048
HALF = DINNER // 2
P = 128
NT = L // 512
KD = DIM // P               # 4 k-tiles over D
MI = (DINNER + HALF) // P   # 12 in_proj M-tiles (8 xc + 4 z)
MX = DINNER // P            # 8 xc tiles
MH = HALF // P              # 4 scan-channel tiles
L2 = 2 * L                  # fused j-pair width
XPAD = 4                    # zero pad cols at the head of xc tiles
EPS = 1e-5

_CACHE = {}


def _build(trace_sim=False):
    nc = bacc.Bacc("TRN2", target_bir_lowering=False, debug=False,
                   num_devices=8)
    dram = {}
    def din(name, shape, dt):
        dram[name] = nc.dram_tensor(name, shape, dt, kind="ExternalInput").ap()
    din("xT", [DIM, L], F32R)
    din("inw", [DIM, P * MI], BF16)
    din("inr1", [2, P * MI], BF16)        # rank-1 lhsT: [wsum; bias]
    din("convd", [P, MX * DCONV * P], BF16)  # diag(conv_w) blocks
    din("convb", [P, MX], F32)
    din("xpw", [DINNER, 64], BF16)
    din("dtpw", [DTRANK, HALF], BF16)
    din("dtb", [P, MH], F32)
    din("dvecd", [P, MH * P], BF16)       # diag(D) blocks
    din("weff", [HALF, DIM], BF16)
    din("onesf", [P, 1], F32R)
    din("ident", [P, P], BF16)
    outT = nc.dram_tensor("outT", [DIM, L], F32, kind="ExternalOutput").ap()
    brows = nc.dram_tensor("brows", [DSTATE, L], BF16).ap()
    crows = nc.dram_tensor("crows", [DSTATE, L], BF16).ap()
    rrow = nc.dram_tensor("rrow", [1, L], BF16).ap()

    with tile.TileContext(nc, trace_sim=trace_sim) as tc, ExitStack() as ctx:
        sb = ctx.enter_context(tc.tile_pool(name="sb", bufs=1))
        ppA = tc.alloc_tile_pool(name="ppA", bufs=2, space="PSUM")

        # ---- weights (split across both HWDGE queues) ----
        inw = [sb.tile([P, P * MI], BF16, tag=f"w{k}", name=f"w{k}") for k in range(KD)]
        for k in range(KD):
            (nc.sync if k % 2 == 0 else nc.scalar).dma_start(
                inw[k][:], dram["inw"][k * P:(k + 1) * P, :])
        inr1 = sb.tile([2, P * MI], BF16, tag="inr1", name="inr1")
        nc.sync.dma_start(inr1[:], dram["inr1"][:])
        convd = sb.tile([P, MX * DCONV * P], BF16, tag="convd", name="convd")
        nc.scalar.dma_start(convd[:], dram["convd"][:])
        convb = sb.tile([P, MX], F32, tag="convb", name="convb")
        nc.sync.dma_start(convb[:], dram["convb"][:])
        xpw = [sb.tile([P, 64], BF16, tag=f"xpw{k}", name=f"xpw{k}") for k in range(MX)]
        for k in range(MX):
            nc.sync.dma_start(xpw[k][:], dram["xpw"][k * P:(k + 1) * P, :])
        dtpw = sb.tile([DTRANK, HALF], BF16, tag="dtpw", name="dtpw")
        nc.sync.dma_start(dtpw[:], dram["dtpw"][:])
        dtb = sb.tile([P, MH], F32, tag="dtb", name="dtb")
        nc.sync.dma_start(dtb[:], dram["dtb"][:])
        dvecd = sb.tile([P, MH * P], BF16, tag="dvecd", name="dvecd")
        nc.scalar.dma_start(dvecd[:], dram["dvecd"][:])
        weff = [sb.tile([P, DIM], BF16, tag=f"wef{k}", name=f"wef{k}") for k in range(MH)]
        for k in range(MH):
            nc.scalar.dma_start(weff[k][:], dram["weff"][k * P:(k + 1) * P, :])
        onesf = sb.tile([P, 1], F32R, tag="onesf", name="onesf")
        nc.sync.dma_start(onesf[:], dram["onesf"][:])
        ident = sb.tile([P, P], BF16, tag="ident", name="ident")
        nc.sync.dma_start(ident[:], dram["ident"][:])
        ceps = sb.tile([1, 1], F32, tag="ceps", name="ceps")
        nc.vector.memset(ceps[:], EPS)
        cone = sb.tile([P, 1], F32, tag="cone", name="cone")
        nc.vector.memset(cone[:], 1.0)

        # ---- phase A: load xT once, LN stats via PE ----
        xt = [sb.tile([P, L], F32R, tag=f"b8{k}", name=f"xt{k}") for k in range(KD)]
        for k in range(KD):
            (nc.sync if k % 2 == 0 else nc.scalar).dma_start(
                xt[k][:], dram["xT"][k * P:(k + 1) * P, :])
        pssum = ppA.tile([1, L], F32, tag="pa", name="st0")
        pssq = ppA.tile([1, L], F32, tag="pa", name="st1")
        for k in range(KD):
            xsq = sb.tile([P, L], F32R, tag=f"b8{4 + k % 2}", name=f"sq{k % 2}")
            nc.scalar.activation(xsq[:], xt[k][:], AF.Square)
            for c in range(NT):
                sl = slice(c * 512, (c + 1) * 512)
                nc.tensor.matmul(pssum[:, sl], onesf[:], xt[k][:, sl],
                                 start=(k == 0), stop=(k == KD - 1))
                nc.tensor.matmul(pssq[:, sl], onesf[:], xsq[:, sl],
                                 start=(k == 0), stop=(k == KD - 1))
        mu = sb.tile([1, L], F32, tag="s0", name="mu")
        m2 = sb.tile([1, L], F32, tag="s1", name="m2")
        nc.scalar.activation(mu[:], pssum[:], AF.Copy, scale=1.0 / DIM)
        nc.scalar.activation(m2[:], pssq[:], AF.Copy, scale=1.0 / DIM)
        mu2 = ppA.tile([1, L], F32, tag="pa", name="mu2")
        nc.vector.tensor_tensor(mu2[:], mu[:], mu[:], OP.mult)
        var = ppA.tile([1, L], F32, tag="pa", name="var")
        nc.vector.tensor_tensor(var[:], m2[:], mu2[:], OP.subtract)
        lnv = ppA.tile([1, L], F32, tag="pa", name="lnv")
        nc.scalar.activation(lnv[:], var[:], AF.Ln, bias=ceps[:])
        rstd = sb.tile([1, L], BF16, tag="s2", name="rstd")
        nc.scalar.activation(rstd[:], lnv[:], AF.Exp, scale=-0.5)
        nc.sync.dma_start(rrow[:], rstd[:])
        r1rhs = sb.tile([2, L], BF16, tag="r1r", name="r1r")
        # row0 = +mu*rstd (host negates wsum); row1 = ones (memset full, row0
        # overwritten after - engines cannot address partition offset 1)
        nc.vector.memset(r1rhs[:], 1.0)
        nc.vector.tensor_tensor(r1rhs[0:1, :], mu[:], rstd[:], OP.mult)
        rstd_b = sb.tile([P, L], BF16, tag="rb", name="rb")
        nc.sync.dma_start(rstd_b[:], rrow[0:1, :].broadcast_to([P, L]))
        xs = [sb.tile([P, L], BF16, tag=f"s4{k}", name=f"xs{k}") for k in range(KD)]
        for k in range(KD):
            nc.vector.tensor_tensor(xs[k][:], xt[k][:], rstd_b[:], OP.mult)

        # ---- phase B+C: in_proj (+rank-1 LN fold) and conv, interleaved ----
        ppA.release()
        ppB = tc.alloc_tile_pool(name="ppB", bufs=2, space="PSUM")   # [128,512]
        ppC = tc.alloc_tile_pool(name="ppC", bufs=2, space="PSUM")   # [128,512] conv
        ppX = tc.alloc_tile_pool(name="ppX", bufs=1, space="PSUM")   # [64,L] xproj
        u2 = [sb.tile([P, L2], BF16, tag=f"u2{h}", name=f"u2{h}") for h in range(2)]
        zs2 = [sb.tile([P, L2], BF16, tag=f"zs{h}", name=f"zs{h}") for h in range(2)]
        px = ppX.tile([64, L], F32, tag="px", name="px")

        xcs = {}
        uos = {}
        for m in range(MI):
            if m < MX:
                xcm = sb.tile([P, L + XPAD], BF16, tag="xc0", name=f"xc{m}")
                nc.vector.memset(xcm[:, 0:XPAD], 0.0)
                xcs[m] = xcm
            for c in range(NT):
                sl = slice(c * 512, (c + 1) * 512)
                pmm = ppB.tile([P, 512], F32, tag="mmb", name="mmb")
                for k in range(KD):
                    nc.tensor.matmul(pmm[:], inw[k][:, m * P:(m + 1) * P],
                                     xs[k][:, sl], start=(k == 0), stop=False)
                nc.tensor.matmul(pmm[:], inr1[:, m * P:(m + 1) * P],
                                 r1rhs[:, sl], start=False, stop=True)
                if m < MX:
                    psl = slice(XPAD + c * 512, XPAD + (c + 1) * 512)
                    if c % 2 == 0:
                        nc.scalar.activation(xcs[m][:, psl], pmm[:], AF.Copy)
                    else:
                        nc.vector.tensor_copy(xcs[m][:, psl], pmm[:])
                else:
                    h = (m - MX) // 2
                    seg = ((m - MX) % 2) * L
                    nc.scalar.activation(zs2[h][:, seg + c * 512:seg + (c + 1) * 512],
                                         pmm[:], AF.Silu)
            if m < MX:
                # depthwise conv: 4 shifted diag-matmuls; SiLU on evacuation.
                # m<4 are this core's scan channels (u2 pair tiles); m 4..7
                # only feed xproj (transient uo tiles).
                if m < 4:
                    udst, uoff = u2[m // 2], (m % 2) * L
                else:
                    uos[m] = sb.tile([P, L], BF16, tag=("rb", "uo1")[m % 2],
                                     name=f"uo{m}")
                    udst, uoff = uos[m], 0
                for c in range(NT):
                    pcv = ppC.tile([P, 512], F32, tag="cv", name="cv")
                    for k in range(DCONV):
                        wsl = slice((m * DCONV + k) * P, (m * DCONV + k + 1) * P)
                        off = XPAD - 3 + k + c * 512
                        nc.tensor.matmul(pcv[:], convd[:, wsl],
                                         xcs[m][:, off:off + 512],
                                         start=(k == 0), stop=(k == DCONV - 1))
                    nc.scalar.activation(
                        udst[:, uoff + c * 512:uoff + (c + 1) * 512],
                        pcv[:], AF.Silu, bias=convb[:, m:m + 1])
                # xproj contribution for this m (after u is ready)
                for c in range(NT):
                    sl = slice(c * 512, (c + 1) * 512)
                    nc.tensor.matmul(px[:, sl], xpw[m][:],
                                     udst[:, uoff + c * 512:uoff + (c + 1) * 512],
                                     start=(m == 0), stop=(m == MX - 1))

        # ---- phase D: dbl, dtproj, softplus, dtu ----
        dbl = sb.tile([64, L], BF16, tag="dbl", name="dbl")
        nc.scalar.activation(dbl[:], px[:], AF.Copy)
        nc.sync.dma_start(brows[:], dbl[DTRANK:DTRANK + DSTATE, :])
        nc.scalar.dma_start(crows[:], dbl[DTRANK + DSTATE:64, :])
        ppX.release()
        ppC.release()
        ppB.release()
        ppD = tc.alloc_tile_pool(name="ppD", bufs=1, space="PSUM")   # [128,L]
        dt2 = [sb.tile([P, L2], BF16, tag=f"dt{h}", name=f"dt{h}") for h in range(2)]
        for h in range(2):
            for s in range(2):
                m = h * 2 + s
                pd = ppD.tile([P, L], F32, tag="pd", name="pd")
                for c in range(NT):
                    sl = slice(c * 512, (c + 1) * 512)
                    nc.tensor.matmul(pd[:, sl], dtpw[:, m * P:(m + 1) * P],
                                     dbl[0:DTRANK, sl], start=True, stop=True)
                seg = slice(s * L, (s + 1) * L)
                nc.scalar.activation(dt2[h][:, seg], pd[:], AF.Exp,
                                     bias=dtb[:, m:m + 1])
        for h in range(2):
            nc.scalar.activation(dt2[h][:, 0:L], dt2[h][:, 0:L], AF.Ln, bias=cone[:])
            nc.scalar.activation(dt2[h][:, L:L2], dt2[h][:, L:L2], AF.Ln, bias=cone[:])
        dtu2 = [sb.tile([P, L2], BF16, tag=f"du{h}", name=f"du{h}") for h in range(2)]
        for h in range(2):
            nc.vector.tensor_tensor(dtu2[h][:], dt2[h][:], u2[h][:], OP.mult)
            # poison the pair boundary so the fused scan's carry dies there
            nc.vector.memset(dt2[h][:, L:L + 1], 30.0)

        # ---- phase E: 2 pair-passes x 16 states ----
        ppD.release()
        ppE = tc.alloc_tile_pool(name="ppE", bufs=1, space="PSUM")   # [128,L2]
        ym2 = [None, None]
        for h in range(2):
            yp = ppE.tile([P, L2], F32, tag="yp", name="yp")
            for n in range(1, DSTATE + 1):
                bn = sb.tile([P, L], BF16, tag=f"s4{n % 2}", name="bn")
                cn = sb.tile([P, L], BF16, tag=f"s4{2 + n % 2}", name="cn")
                nc.sync.dma_start(bn[:], brows[n - 1:n, :].broadcast_to([P, L]))
                nc.scalar.dma_start(cn[:], crows[n - 1:n, :].broadcast_to([P, L]))
                dA = sb.tile([P, L2], BF16, tag=f"b8{n % 2}", name="dA")
                nc.scalar.activation(dA[:], dt2[h][:], AF.Exp, scale=float(-n))
                dBu = sb.tile([P, L2], BF16, tag=f"b8{2 + n % 2}", name="dBu")
                nc.vector.tensor_tensor(dBu[:, 0:L], dtu2[h][:, 0:L], bn[:], OP.mult)
                nc.vector.tensor_tensor(dBu[:, L:L2], dtu2[h][:, L:L2], bn[:], OP.mult)
                hh = sb.tile([P, L2], BF16, tag=f"b8{4 + n % 2}", name="hh")
                nc.vector.tensor_tensor_scan(hh[:], dA[:], dBu[:], 0.0,
                                             OP.mult, OP.add)
                hc = sb.tile([P, L2], BF16, tag=f"b8{2 + n % 2}", name="hc")
                nc.vector.tensor_tensor(hc[:, 0:L], hh[:, 0:L], cn[:], OP.mult)
                nc.vector.tensor_tensor(hc[:, L:L2], hh[:, L:L2], cn[:], OP.mult)
                for i in range(2 * NT):
                    sl = slice(i * 512, (i + 1) * 512)
                    nc.tensor.matmul(yp[:, sl], ident[:], hc[:, sl],
                                     start=(n == 1), stop=False)
            # += diag(D) @ u  (D*u term), closes each slice's accumulation
            for s in range(2):
                m = h * 2 + s
                for c in range(NT):
                    sl = slice(s * L + c * 512, s * L + (c + 1) * 512)
                    nc.tensor.matmul(yp[:, sl], dvecd[:, m * P:(m + 1) * P],
                                     u2[h][:, sl], start=False, stop=True)
            yps = sb.tile([P, L2], BF16, tag=f"b8{h}", name="yps")
            nc.scalar.activation(yps[:, 0:L], yp[:, 0:L], AF.Copy)
            nc.scalar.activation(yps[:, L:L2], yp[:, L:L2], AF.Copy)
            ym2[h] = sb.tile([P, L2], BF16, tag=f"u2{h}", name=f"ym{h}")
            nc.vector.tensor_tensor(ym2[h][:], yps[:], zs2[h][:], OP.mult)

        # ---- phase F: out_proj (weff = fuse @ out_w, premultiplied) ----
        ppE.release()
        ppF = tc.alloc_tile_pool(name="ppF", bufs=2, space="PSUM")
        for half in range(2):
            po = [ppF.tile([P, L], F32, tag="po", name="po") for _ in range(2)]
            for k in range(MH):
                h, seg = k // 2, (k % 2) * L
                for j in range(2):
                    mo = half * 2 + j
                    for c in range(NT):
                        sl = slice(c * 512, (c + 1) * 512)
                        nc.tensor.matmul(po[j][:, sl],
                                         weff[k][:, mo * P:(mo + 1) * P],
                                         ym2[h][:, seg + c * 512:seg + (c + 1) * 512],
                                         start=(k == 0), stop=(k == MH - 1))
            for j in range(2):
                mo = half * 2 + j
                for c in range(NT):
                    sl = slice(c * 512, (c + 1) * 512)
                    ev = sb.tile([P, 512], F32, tag="ev0", name="ev")
                    if (j * NT + c) % 2 == 0:
                        nc.scalar.activation(ev[:], po[j][:, sl], AF.Copy)
                    else:
                        nc.vector.tensor_copy(ev[:], po[j][:, sl])
                    (nc.sync if c % 2 == 0 else nc.scalar).dma_start(
                        outT[mo * P:(mo + 1) * P, sl], ev[:])
        ppF.release()
    nc.compile()
    return nc


def _host_prep(inputs):
    f32 = np.float32
    x = np.asarray(inputs["x"], f32)
    ln_g = np.asarray(inputs["ln_g"], f32); ln_b = np.asarray(inputs["ln_b"], f32)
    in_w = np.asarray(inputs["in_w"], f32)
    conv_w = np.asarray(inputs["conv_w"], f32); conv_b = np.asarray(inputs["conv_b"], f32)
    xproj_w = np.asarray(inputs["xproj_w"], f32); dtproj_w = np.asarray(inputs["dtproj_w"], f32)
    dt_bias = np.asarray(inputs["dt_bias"], f32)
    D = np.asarray(inputs["D"], f32)
    out_w = np.asarray(inputs["out_w"], f32)
    fuse_w = np.asarray(inputs["fuse_w"], f32)

    maps = []
    for p in range(4):
        dir_, b = p // 2, p % 2
        W = in_w[dir_] * ln_g[None, :]          # [2*Di, D], LN gain folded
        in_bias_full = in_w[dir_] @ ln_b        # LN bias folded
        Weff_out = fuse_w[:, dir_ * DIM:(dir_ + 1) * DIM] @ out_w[dir_]
        xb = x[b] if dir_ == 0 else x[b, ::-1]
        for half in range(2):
            sl = slice(half * HALF, (half + 1) * HALF)
            # permute xc channels so this core's scan channels are rows 0..511
            perm = np.concatenate([np.arange(half * HALF, (half + 1) * HALF),
                                   np.arange((1 - half) * HALF, (2 - half) * HALF)])
            rows = np.concatenate([perm, DINNER + np.arange(half * HALF, (half + 1) * HALF)])
            Wr = W[rows]
            convd = np.zeros((P, MX * DCONV * P), f32)
            cw = conv_w[dir_][perm]             # [DINNER, DCONV]
            for j in range(MX):
                for k in range(DCONV):
                    blk = (j * DCONV + k) * P
                    convd[:, blk:blk + P] = np.diag(cw[j * P:(j + 1) * P, k])
            dvecd = np.zeros((P, MH * P), f32)
            dv = D[dir_, sl]
            for mm in range(MH):
                dvecd[:, mm * P:(mm + 1) * P] = np.diag(dv[mm * P:(mm + 1) * P])
            m = dict(
                xT=np.ascontiguousarray(xb.T),
                inw=np.ascontiguousarray(Wr.T.astype(BF)),
                inr1=np.ascontiguousarray(
                    np.stack([-Wr.sum(1), in_bias_full[rows]]).astype(BF)),
                convd=convd.astype(BF),
                convb=np.ascontiguousarray(conv_b[dir_][perm].reshape(MX, P).T),
                xpw=np.ascontiguousarray(xproj_w[dir_][:, perm].T.astype(BF)),
                dtpw=np.ascontiguousarray(dtproj_w[dir_, sl].T.astype(BF)),
                dtb=np.ascontiguousarray(dt_bias[dir_, sl].reshape(MH, P).T),
                dvecd=dvecd.astype(BF),
                weff=np.ascontiguousarray(Weff_out[:, sl].T.astype(BF)),
                onesf=np.ones((P, 1), np.float32),
                ident=np.eye(P, dtype=np.float32).astype(BF),
            )
            maps.append(m)
    return maps


def kernel(**inputs):
    if "nc" not in _CACHE:
        _CACHE["nc"] = _build()
    nc = _CACHE["nc"]
    maps = _host_prep(inputs)
    res = run_bass_kernel_spmd(nc, maps, list(range(8)))
    x = np.asarray(inputs["x"], np.float32)
    fuse_b = np.asarray(inputs["fuse_b"], np.float32)
    out = x + fuse_b[None, None, :]
    for p in range(4):
        dir_, b = p // 2, p % 2
        for half in range(2):
            pt = np.asarray(res.results[p * 2 + half]["outT"], np.float32).T
            if dir_ == 1:
                pt = pt[::-1]
            out[b] += pt
    return out.astype(np.float32)


# revision 9
# speedup vs baseline: 1.1866x; 1.1866x over previous
"""BiMamba layer on 8 TRN2 NeuronCores — v2.

Sharding: 8 cores = 4 (dir,batch) pairs x 2 halves of d_inner; host flips
the sequence for the backward direction, transposes to [channel, token]
layout, and sums the 4 partial outputs per batch + residual at the end.

v2 changes vs baseline (driven by HW microbenchmarks):
  - LN folded into in_proj: xs = x*rstd (DVE); the -mu*rstd and bias
    terms ride a K=2 rank-1 matmul accumulated into the in_proj PSUM.
    xT loaded once and kept resident (baseline streamed it twice).
  - in_proj in bf16 (was f32r).
  - depthwise conv on the PE as 4 shifted diag-matmuls (was DVE
    scalar_tensor_tensor at 1x); SiLU fused into the PSUM evacuation
    via the ACT Silu table (kills sigmoid+mult on DVE).
  - z-SiLU fused into the in_proj z evacuation (ACT Silu).
  - scan phase: j-pairs fused into [128, 2L] tiles: one exp + one scan
    per (n,pair); a poison column (dt=30) at the segment boundary kills
    the scan carry between the two channel tiles. B/C broadcasts ride
    two HWDGE queues (sync=B, scalar=C), double-buffered.
  - u*D folded into the scan PSUM via a diag(D) matmul.
GPSIMD left idle on purpose: its SBUF port is shared with the DVE and
concurrent use halves both engines' throughput (measured).
"""
import sys
sys.path.insert(0, '/opt/trn_rl_repo')
import numpy as np
import ml_dtypes
from contextlib import ExitStack

import concourse.bass as bass
import concourse.tile as tile
from concourse import bacc, mybir
from concourse.bass_utils import run_bass_kernel_spmd

AF = mybir.ActivationFunctionType
OP = mybir.AluOpType
F32, BF16, F32R = mybir.dt.float32, mybir.dt.bfloat16, mybir.dt.float32r
BF = ml_dtypes.bfloat16

DIM, DSTATE, DCONV, DINNER, DTRANK, B, L = 512, 16, 4, 1024, 32, 2, 2048
HALF = DINNER // 2
P = 128
NT = L // 512
KD = DIM // P               # 4 k-tiles over D
MI = (DINNER + HALF) // P   # 12 in_proj M-tiles (8 xc + 4 z)
MX = DINNER // P            # 8 xc tiles
MH = HALF // P              # 4 scan-channel tiles
L2 = 2 * L                  # fused j-pair width
XPAD = 4                    # zero pad cols at the head of xc tiles
EPS = 1e-5

_CACHE = {}


def _build(trace_sim=False):
    nc = bacc.Bacc("TRN2", target_bir_lowering=False, debug=False,
                   num_devices=8)
    dram = {}
    def din(name, shape, dt):
        dram[name] = nc.dram_tensor(name, shape, dt, kind="ExternalInput").ap()
    din("xT", [DIM, L], F32R)
    din("inw", [DIM, P * MI], BF16)
    din("inr1", [2, P * MI], BF16)        # rank-1 lhsT: [wsum; bias]
    din("convd", [P, MX * DCONV * P], BF16)  # diag(conv_w) blocks
    din("convb", [P, MX], F32)
    din("xpw", [DINNER, 64], BF16)
    din("dtpw", [DTRANK, HALF], BF16)
    din("dtb", [P, MH], F32)
    din("dvecd", [P, MH * P], BF16)       # diag(D) blocks
    din("weff", [HALF, DIM], BF16)
    din("onesf", [P, 1], F32R)
    din("ident", [P, P], BF16)
    outT = nc.dram_tensor("outT", [DIM, L], F32, kind="ExternalOutput").ap()
    brows = nc.dram_tensor("brows", [DSTATE, L], BF16).ap()
    crows = nc.dram_tensor("crows", [DSTATE, L], BF16).ap()
    rrow = nc.dram_tensor("rrow", [1, L], BF16).ap()

    with tile.TileContext(nc, trace_sim=trace_sim) as tc, ExitStack() as ctx:
        sb = ctx.enter_context(tc.tile_pool(name="sb", bufs=1))
        ppA = tc.alloc_tile_pool(name="ppA", bufs=2, space="PSUM")

        # ---- weights (split across both HWDGE queues) ----
        inw = [sb.tile([P, P * MI], BF16, tag=f"w{k}", name=f"w{k}") for k in range(KD)]
        for k in range(KD):
            (nc.sync if k % 2 == 0 else nc.scalar).dma_start(
                inw[k][:], dram["inw"][k * P:(k + 1) * P, :])
        inr1 = sb.tile([2, P * MI], BF16, tag="inr1", name="inr1")
        nc.sync.dma_start(inr1[:], dram["inr1"][:])
        convd = sb.tile([P, MX * DCONV * P], BF16, tag="convd", name="convd")
        nc.scalar.dma_start(convd[:], dram["convd"][:])
        convb = sb.tile([P, MX], F32, tag="convb", name="convb")
        nc.sync.dma_start(convb[:], dram["convb"][:])
        xpw = [sb.tile([P, 64], BF16, tag=f"xpw{k}", name=f"xpw{k}") for k in range(MX)]
        for k in range(MX):
            nc.sync.dma_start(xpw[k][:], dram["xpw"][k * P:(k + 1) * P, :])
        dtpw = sb.tile([DTRANK, HALF], BF16, tag="dtpw", name="dtpw")
        nc.sync.dma_start(dtpw[:], dram["dtpw"][:])
        dtb = sb.tile([P, MH], F32, tag="dtb", name="dtb")
        nc.sync.dma_start(dtb[:], dram["dtb"][:])
        dvecd = sb.tile([P, MH * P], BF16, tag="dvecd", name="dvecd")
        nc.scalar.dma_start(dvecd[:], dram["dvecd"][:])
        weff = [sb.tile([P, DIM], BF16, tag=f"wef{k}", name=f"wef{k}") for k in range(MH)]
        for k in range(MH):
            nc.scalar.dma_start(weff[k][:], dram["weff"][k * P:(k + 1) * P, :])
        onesf = sb.tile([P, 1], F32R, tag="onesf", name="onesf")
        nc.sync.dma_start(onesf[:], dram["onesf"][:])
        ident = sb.tile([P, P], BF16, tag="ident", name="ident")
        nc.sync.dma_start(ident[:], dram["ident"][:])
        ceps = sb.tile([1, 1], F32, tag="ceps", name="ceps")
        nc.vector.memset(ceps[:], EPS)
        cone = sb.tile([P, 1], F32, tag="cone", name="cone")
        nc.vector.memset(cone[:], 1.0)

        # ---- phase A: load xT once, LN stats via PE ----
        xt = [sb.tile([P, L], F32R, tag=f"b8{k}", name=f"xt{k}") for k in range(KD)]
        for k in range(KD):
            (nc.sync if k % 2 == 0 else nc.scalar).dma_start(
                xt[k][:], dram["xT"][k * P:(k + 1) * P, :])
        pssum = ppA.tile([1, L], F32, tag="pa", name="st0")
        pssq = ppA.tile([1, L], F32, tag="pa", name="st1")
        for k in range(KD):
            xsq = sb.tile([P, L], F32R, tag=f"b8{4 + k % 2}", name=f"sq{k % 2}")
            nc.scalar.activation(xsq[:], xt[k][:], AF.Square)
            for c in range(NT):
                sl = slice(c * 512, (c + 1) * 512)
                nc.tensor.matmul(pssum[:, sl], onesf[:], xt[k][:, sl],
                                 start=(k == 0), stop=(k == KD - 1))
                nc.tensor.matmul(pssq[:, sl], onesf[:], xsq[:, sl],
                                 start=(k == 0), stop=(k == KD - 1))
        mu = sb.tile([1, L], F32, tag="s0", name="mu")
        m2 = sb.tile([1, L], F32, tag="s1", name="m2")
        nc.scalar.activation(mu[:], pssum[:], AF.Copy, scale=1.0 / DIM)
        nc.scalar.activation(m2[:], pssq[:], AF.Copy, scale=1.0 / DIM)
        mu2 = ppA.tile([1, L], F32, tag="pa", name="mu2")
        nc.vector.tensor_tensor(mu2[:], mu[:], mu[:], OP.mult)
        var = ppA.tile([1, L], F32, tag="pa", name="var")
        nc.vector.tensor_tensor(var[:], m2[:], mu2[:], OP.subtract)
        lnv = ppA.tile([1, L], F32, tag="pa", name="lnv")
        nc.scalar.activation(lnv[:], var[:], AF.Ln, bias=ceps[:])
        rstd = sb.tile([1, L], BF16, tag="s2", name="rstd")
        nc.scalar.activation(rstd[:], lnv[:], AF.Exp, scale=-0.5)
        nc.sync.dma_start(rrow[:], rstd[:])
        r1rhs = sb.tile([2, L], BF16, tag="r1r", name="r1r")
        # row0 = +mu*rstd (host negates wsum); row1 = ones (memset full, row0
        # overwritten after - engines cannot address partition offset 1)
        nc.vector.memset(r1rhs[:], 1.0)
        nc.vector.tensor_tensor(r1rhs[0:1, :], mu[:], rstd[:], OP.mult)
        rstd_b = sb.tile([P, L], BF16, tag="rb", name="rb")
        nc.sync.dma_start(rstd_b[:], rrow[0:1, :].broadcast_to([P, L]))
        xs = [sb.tile([P, L], BF16, tag=f"s4{k}", name=f"xs{k}") for k in range(KD)]
        for k in range(KD):
            nc.vector.tensor_tensor(xs[k][:], xt[k][:], rstd_b[:], OP.mult)

        # ---- phase B+C: in_proj (+rank-1 LN fold) and conv, interleaved ----
        ppA.release()
        ppB = tc.alloc_tile_pool(name="ppB", bufs=1, space="PSUM")   # [128,1024]
        ppC = tc.alloc_tile_pool(name="ppC", bufs=1, space="PSUM")   # [128,1024] conv
        ppX = tc.alloc_tile_pool(name="ppX", bufs=1, space="PSUM")   # [64,L] xproj
        u2 = [sb.tile([P, L2], BF16, tag=f"u2{h}", name=f"u2{h}") for h in range(2)]
        zs2 = [sb.tile([P, L2], BF16, tag=f"zs{h}", name=f"zs{h}") for h in range(2)]
        px = ppX.tile([64, L], F32, tag="px", name="px")

        xcs = {}
        uos = {}
        for m in range(MI):
            if m < MX:
                xcm = sb.tile([P, L + XPAD], BF16, tag="xc0", name=f"xc{m}")
                nc.vector.memset(xcm[:, 0:XPAD], 0.0)
                xcs[m] = xcm
            for c in range(NT):
                sl = slice(c * 512, (c + 1) * 512)
                pmm = ppB.tile([P, 512], F32, tag="mmb", name="mmb")
                for k in range(KD):
                    nc.tensor.matmul(pmm[:], inw[k][:, m * P:(m + 1) * P],
                                     xs[k][:, sl], start=(k == 0), stop=False)
                nc.tensor.matmul(pmm[:], inr1[:, m * P:(m + 1) * P],
                                 r1rhs[:, sl], start=False, stop=True)
                if m < MX:
                    psl = slice(XPAD + c * 512, XPAD + (c + 1) * 512)
                    if c % 2 == 0:
                        nc.scalar.activation(xcs[m][:, psl], pmm[:], AF.Copy)
                    else:
                        nc.vector.tensor_copy(xcs[m][:, psl], pmm[:])
                else:
                    h = (m - MX) // 2
                    seg = ((m - MX) % 2) * L
                    nc.scalar.activation(zs2[h][:, seg + c * 512:seg + (c + 1) * 512],
                                         pmm[:], AF.Silu)
            if m < MX:
                # depthwise conv: 4 shifted diag-matmuls; SiLU on evacuation.
                # m<4 are this core's scan channels (u2 pair tiles); m 4..7
                # only feed xproj (transient uo tiles).
                if m < 4:
                    udst, uoff = u2[m // 2], (m % 2) * L
                else:
                    uos[m] = sb.tile([P, L], BF16, tag=("rb", "uo1")[m % 2],
                                     name=f"uo{m}")
                    udst, uoff = uos[m], 0
                for c in range(NT):
                    pcv = ppC.tile([P, 512], F32, tag="cv", name="cv")
                    for k in range(DCONV):
                        wsl = slice((m * DCONV + k) * P, (m * DCONV + k + 1) * P)
                        off = XPAD - 3 + k + c * 512
                        nc.tensor.matmul(pcv[:], convd[:, wsl],
                                         xcs[m][:, off:off + 512],
                                         start=(k == 0), stop=(k == DCONV - 1))
                    nc.scalar.activation(
                        udst[:, uoff + c * 512:uoff + (c + 1) * 512],
                        pcv[:], AF.Silu, bias=convb[:, m:m + 1])
                # xproj contribution for this m (after u is ready)
                for c in range(NT):
                    sl = slice(c * 512, (c + 1) * 512)
                    nc.tensor.matmul(px[:, sl], xpw[m][:],
                                     udst[:, uoff + c * 512:uoff + (c + 1) * 512],
                                     start=(m == 0), stop=(m == MX - 1))

        # ---- phase D: dbl, dtproj, softplus, dtu ----
        dbl = sb.tile([64, L], BF16, tag="dbl", name="dbl")
        nc.scalar.activation(dbl[:], px[:], AF.Copy)
        nc.sync.dma_start(brows[:], dbl[DTRANK:DTRANK + DSTATE, :])
        nc.scalar.dma_start(crows[:], dbl[DTRANK + DSTATE:64, :])
        ppX.release()
        ppC.release()
        ppB.release()
        ppD = tc.alloc_tile_pool(name="ppD", bufs=1, space="PSUM")   # [128,L]
        dt2 = [sb.tile([P, L2], BF16, tag=f"dt{h}", name=f"dt{h}") for h in range(2)]
        for h in range(2):
            for s in range(2):
                m = h * 2 + s
                pd = ppD.tile([P, L], F32, tag="pd", name="pd")
                for c in range(NT):
                    sl = slice(c * 512, (c + 1) * 512)
                    nc.tensor.matmul(pd[:, sl], dtpw[:, m * P:(m + 1) * P],
                                     dbl[0:DTRANK, sl], start=True, stop=True)
                seg = slice(s * L, (s + 1) * L)
                nc.scalar.activation(dt2[h][:, seg], pd[:], AF.Exp,
                                     bias=dtb[:, m:m + 1])
        for h in range(2):
            nc.scalar.activation(dt2[h][:, 0:L], dt2[h][:, 0:L], AF.Ln, bias=cone[:])
            nc.scalar.activation(dt2[h][:, L:L2], dt2[h][:, L:L2], AF.Ln, bias=cone[:])
        dtu2 = [sb.tile([P, L2], BF16, tag=f"du{h}", name=f"du{h}") for h in range(2)]
        for h in range(2):
            nc.vector.tensor_tensor(dtu2[h][:], dt2[h][:], u2[h][:], OP.mult)
            # poison the pair boundary so the fused scan's carry dies there
            nc.vector.memset(dt2[h][:, L:L + 1], 30.0)

        # ---- phase E: 2 pair-passes x 16 states ----
        ppD.release()
        ppE = tc.alloc_tile_pool(name="ppE", bufs=1, space="PSUM")   # [128,L2]
        ym2 = [None, None]
        for h in range(2):
            yp = ppE.tile([P, L2], F32, tag="yp", name="yp")
            for n in range(1, DSTATE + 1):
                bn = sb.tile([P, L], BF16, tag=f"s4{n % 2}", name="bn")
                cn = sb.tile([P, L], BF16, tag=f"s4{2 + n % 2}", name="cn")
                nc.sync.dma_start(bn[:], brows[n - 1:n, :].broadcast_to([P, L]))
                nc.scalar.dma_start(cn[:], crows[n - 1:n, :].broadcast_to([P, L]))
                dA = sb.tile([P, L2], BF16, tag=f"b8{n % 2}", name="dA")
                nc.scalar.activation(dA[:], dt2[h][:], AF.Exp, scale=float(-n))
                dBu = sb.tile([P, L2], BF16, tag=f"b8{2 + n % 2}", name="dBu")
                nc.vector.tensor_tensor(dBu[:, 0:L], dtu2[h][:, 0:L], bn[:], OP.mult)
                nc.vector.tensor_tensor(dBu[:, L:L2], dtu2[h][:, L:L2], bn[:], OP.mult)
                hh = sb.tile([P, L2], BF16, tag=f"b8{4 + n % 2}", name="hh")
                nc.vector.tensor_tensor_scan(hh[:], dA[:], dBu[:], 0.0,
                                             OP.mult, OP.add)
                hc = sb.tile([P, L2], BF16, tag=f"b8{2 + n % 2}", name="hc")
                nc.vector.tensor_tensor(hc[:, 0:L], hh[:, 0:L], cn[:], OP.mult)
                nc.vector.tensor_tensor(hc[:, L:L2], hh[:, L:L2], cn[:], OP.mult)
                for i in range(2 * NT):
                    sl = slice(i * 512, (i + 1) * 512)
                    nc.tensor.matmul(yp[:, sl], ident[:], hc[:, sl],
                                     start=(n == 1), stop=False)
            # += diag(D) @ u  (D*u term), closes each slice's accumulation
            for s in range(2):
                m = h * 2 + s
                for c in range(NT):
                    sl = slice(s * L + c * 512, s * L + (c + 1) * 512)
                    nc.tensor.matmul(yp[:, sl], dvecd[:, m * P:(m + 1) * P],
                                     u2[h][:, sl], start=False, stop=True)
            yps = sb.tile([P, L2], BF16, tag=f"b8{h}", name="yps")
            nc.scalar.activation(yps[:, 0:L], yp[:, 0:L], AF.Copy)
            nc.scalar.activation(yps[:, L:L2], yp[:, L:L2], AF.Copy)
            ym2[h] = sb.tile([P, L2], BF16, tag=f"u2{h}", name=f"ym{h}")
            nc.vector.tensor_tensor(ym2[h][:], yps[:], zs2[h][:], OP.mult)

        # ---- phase F: out_proj (weff = fuse @ out_w, premultiplied) ----
        ppE.release()
        ppF = tc.alloc_tile_pool(name="ppF", bufs=2, space="PSUM")
        for half in range(2):
            po = [ppF.tile([P, L], F32, tag="po", name="po") for _ in range(2)]
            for k in range(MH):
                h, seg = k // 2, (k % 2) * L
                for j in range(2):
                    mo = half * 2 + j
                    for c in range(NT):
                        sl = slice(c * 512, (c + 1) * 512)
                        nc.tensor.matmul(po[j][:, sl],
                                         weff[k][:, mo * P:(mo + 1) * P],
                                         ym2[h][:, seg + c * 512:seg + (c + 1) * 512],
                                         start=(k == 0), stop=(k == MH - 1))
            for j in range(2):
                mo = half * 2 + j
                for c in range(NT):
                    sl = slice(c * 512, (c + 1) * 512)
                    ev = sb.tile([P, 512], F32, tag=f"ev{(j * NT + c) % 2}", name="ev")
                    if (j * NT + c) % 2 == 0:
                        nc.scalar.activation(ev[:], po[j][:, sl], AF.Copy)
                    else:
                        nc.vector.tensor_copy(ev[:], po[j][:, sl])
                    (nc.sync if c % 2 == 0 else nc.scalar).dma_start(
                        outT[mo * P:(mo + 1) * P, sl], ev[:])
        ppF.release()
    nc.compile()
    return nc


def _host_prep(inputs):
    f32 = np.float32
    x = np.asarray(inputs["x"], f32)
    ln_g = np.asarray(inputs["ln_g"], f32); ln_b = np.asarray(inputs["ln_b"], f32)
    in_w = np.asarray(inputs["in_w"], f32)
    conv_w = np.asarray(inputs["conv_w"], f32); conv_b = np.asarray(inputs["conv_b"], f32)
    xproj_w = np.asarray(inputs["xproj_w"], f32); dtproj_w = np.asarray(inputs["dtproj_w"], f32)
    dt_bias = np.asarray(inputs["dt_bias"], f32)
    D = np.asarray(inputs["D"], f32)
    out_w = np.asarray(inputs["out_w"], f32)
    fuse_w = np.asarray(inputs["fuse_w"], f32)

    maps = []
    for p in range(4):
        dir_, b = p // 2, p % 2
        W = in_w[dir_] * ln_g[None, :]          # [2*Di, D], LN gain folded
        in_bias_full = in_w[dir_] @ ln_b        # LN bias folded
        Weff_out = fuse_w[:, dir_ * DIM:(dir_ + 1) * DIM] @ out_w[dir_]
        xb = x[b] if dir_ == 0 else x[b, ::-1]
        for half in range(2):
            sl = slice(half * HALF, (half + 1) * HALF)
            # permute xc channels so this core's scan channels are rows 0..511
            perm = np.concatenate([np.arange(half * HALF, (half + 1) * HALF),
                                   np.arange((1 - half) * HALF, (2 - half) * HALF)])
            rows = np.concatenate([perm, DINNER + np.arange(half * HALF, (half + 1) * HALF)])
            Wr = W[rows]
            convd = np.zeros((P, MX * DCONV * P), f32)
            cw = conv_w[dir_][perm]             # [DINNER, DCONV]
            for j in range(MX):
                for k in range(DCONV):
                    blk = (j * DCONV + k) * P
                    convd[:, blk:blk + P] = np.diag(cw[j * P:(j + 1) * P, k])
            dvecd = np.zeros((P, MH * P), f32)
            dv = D[dir_, sl]
            for mm in range(MH):
                dvecd[:, mm * P:(mm + 1) * P] = np.diag(dv[mm * P:(mm + 1) * P])
            m = dict(
                xT=np.ascontiguousarray(xb.T),
                inw=np.ascontiguousarray(Wr.T.astype(BF)),
                inr1=np.ascontiguousarray(
                    np.stack([-Wr.sum(1), in_bias_full[rows]]).astype(BF)),
                convd=convd.astype(BF),
                convb=np.ascontiguousarray(conv_b[dir_][perm].reshape(MX, P).T),
                xpw=np.ascontiguousarray(xproj_w[dir_][:, perm].T.astype(BF)),
                dtpw=np.ascontiguousarray(dtproj_w[dir_, sl].T.astype(BF)),
                dtb=np.ascontiguousarray(dt_bias[dir_, sl].reshape(MH, P).T),
                dvecd=dvecd.astype(BF),
                weff=np.ascontiguousarray(Weff_out[:, sl].T.astype(BF)),
                onesf=np.ones((P, 1), np.float32),
                ident=np.eye(P, dtype=np.float32).astype(BF),
            )
            maps.append(m)
    return maps


def kernel(**inputs):
    if "nc" not in _CACHE:
        _CACHE["nc"] = _build()
    nc = _CACHE["nc"]
    maps = _host_prep(inputs)
    res = run_bass_kernel_spmd(nc, maps, list(range(8)))
    x = np.asarray(inputs["x"], np.float32)
    fuse_b = np.asarray(inputs["fuse_b"], np.float32)
    out = x + fuse_b[None, None, :]
    for p in range(4):
        dir_, b = p // 2, p % 2
        for half in range(2):
            pt = np.asarray(res.results[p * 2 + half]["outT"], np.float32).T
            if dir_ == 1:
                pt = pt[::-1]
            out[b] += pt
    return out.astype(np.float32)


# revision 10
# speedup vs baseline: 1.2546x; 1.0573x over previous
"""BiMamba layer on 8 TRN2 NeuronCores — v2.

Sharding: 8 cores = 4 (dir,batch) pairs x 2 halves of d_inner; host flips
the sequence for the backward direction, transposes to [channel, token]
layout, and sums the 4 partial outputs per batch + residual at the end.

v2 changes vs baseline (driven by HW microbenchmarks):
  - LN folded into in_proj: xs = x*rstd (DVE); the -mu*rstd and bias
    terms ride a K=2 rank-1 matmul accumulated into the in_proj PSUM.
    xT loaded once and kept resident (baseline streamed it twice).
  - in_proj in bf16 (was f32r).
  - depthwise conv on the PE as 4 shifted diag-matmuls (was DVE
    scalar_tensor_tensor at 1x); SiLU fused into the PSUM evacuation
    via the ACT Silu table (kills sigmoid+mult on DVE).
  - z-SiLU fused into the in_proj z evacuation (ACT Silu).
  - scan phase: j-pairs fused into [128, 2L] tiles: one exp + one scan
    per (n,pair); a poison column (dt=30) at the segment boundary kills
    the scan carry between the two channel tiles. B/C broadcasts ride
    two HWDGE queues (sync=B, scalar=C), double-buffered.
  - u*D folded into the scan PSUM via a diag(D) matmul.
GPSIMD left idle on purpose: its SBUF port is shared with the DVE and
concurrent use halves both engines' throughput (measured).
"""
import sys
sys.path.insert(0, '/opt/trn_rl_repo')
import numpy as np
import ml_dtypes
from contextlib import ExitStack

import concourse.bass as bass
import concourse.tile as tile
from concourse import bacc, mybir
from concourse.bass_utils import run_bass_kernel_spmd

AF = mybir.ActivationFunctionType
OP = mybir.AluOpType
F32, BF16, F32R = mybir.dt.float32, mybir.dt.bfloat16, mybir.dt.float32r
BF = ml_dtypes.bfloat16

DIM, DSTATE, DCONV, DINNER, DTRANK, B, L = 512, 16, 4, 1024, 32, 2, 2048
HALF = DINNER // 2
P = 128
NT = L // 512
KD = DIM // P               # 4 k-tiles over D
MI = (DINNER + HALF) // P   # 12 in_proj M-tiles (8 xc + 4 z)
MX = DINNER // P            # 8 xc tiles
MH = HALF // P              # 4 scan-channel tiles
L2 = 2 * L                  # fused j-pair width
XPAD = 4                    # zero pad cols at the head of xc tiles
EPS = 1e-5

_CACHE = {}


def _build(trace_sim=False):
    nc = bacc.Bacc("TRN2", target_bir_lowering=False, debug=False,
                   num_devices=8)
    dram = {}
    def din(name, shape, dt):
        dram[name] = nc.dram_tensor(name, shape, dt, kind="ExternalInput").ap()
    din("xT", [DIM, L], BF16)
    din("inw", [DIM, P * MI], BF16)
    din("inr1", [2, P * MI], BF16)        # rank-1 lhsT: [wsum; bias]
    din("convd", [P, MX * DCONV * P], BF16)  # diag(conv_w) blocks
    din("convb", [P, MX], F32)
    din("xpw", [DINNER, 64], BF16)
    din("dtpw", [DTRANK, HALF], BF16)
    din("dtb", [P, MH], F32)
    din("dvecd", [P, MH * P], BF16)       # diag(D) blocks
    din("weff", [HALF, DIM], BF16)
    din("onesf", [P, 1], BF16)
    din("ident", [P, P], BF16)
    outT = nc.dram_tensor("outT", [DIM, L], F32, kind="ExternalOutput").ap()
    brows = nc.dram_tensor("brows", [DSTATE, L], BF16).ap()
    crows = nc.dram_tensor("crows", [DSTATE, L], BF16).ap()
    rrow = nc.dram_tensor("rrow", [1, L], BF16).ap()

    with tile.TileContext(nc, trace_sim=trace_sim) as tc, ExitStack() as ctx:
        sb = ctx.enter_context(tc.tile_pool(name="sb", bufs=1))
        ppA = tc.alloc_tile_pool(name="ppA", bufs=2, space="PSUM")

        # ---- weights (split across both HWDGE queues) ----
        inw = [sb.tile([P, P * MI], BF16, tag=f"w{k}", name=f"w{k}") for k in range(KD)]
        for k in range(KD):
            (nc.sync if k % 2 == 0 else nc.scalar).dma_start(
                inw[k][:], dram["inw"][k * P:(k + 1) * P, :])
        inr1 = sb.tile([2, P * MI], BF16, tag="inr1", name="inr1")
        nc.sync.dma_start(inr1[:], dram["inr1"][:])
        convd = sb.tile([P, MX * DCONV * P], BF16, tag="convd", name="convd")
        nc.scalar.dma_start(convd[:], dram["convd"][:])
        convb = sb.tile([P, MX], F32, tag="convb", name="convb")
        nc.sync.dma_start(convb[:], dram["convb"][:])
        xpw = [sb.tile([P, 64], BF16, tag=f"xpw{k}", name=f"xpw{k}") for k in range(MX)]
        for k in range(MX):
            nc.sync.dma_start(xpw[k][:], dram["xpw"][k * P:(k + 1) * P, :])
        dtpw = sb.tile([DTRANK, HALF], BF16, tag="dtpw", name="dtpw")
        nc.sync.dma_start(dtpw[:], dram["dtpw"][:])
        dtb = sb.tile([P, MH], F32, tag="dtb", name="dtb")
        nc.sync.dma_start(dtb[:], dram["dtb"][:])
        dvecd = sb.tile([P, MH * P], BF16, tag="dvecd", name="dvecd")
        nc.scalar.dma_start(dvecd[:], dram["dvecd"][:])
        weff = [sb.tile([P, DIM], BF16, tag=f"wef{k}", name=f"wef{k}") for k in range(MH)]
        for k in range(MH):
            nc.scalar.dma_start(weff[k][:], dram["weff"][k * P:(k + 1) * P, :])
        onesf = sb.tile([P, 1], BF16, tag="onesf", name="onesf")
        nc.sync.dma_start(onesf[:], dram["onesf"][:])
        ident = sb.tile([P, P], BF16, tag="ident", name="ident")
        nc.sync.dma_start(ident[:], dram["ident"][:])
        ceps = sb.tile([1, 1], F32, tag="ceps", name="ceps")
        nc.vector.memset(ceps[:], EPS)
        cone = sb.tile([P, 1], F32, tag="cone", name="cone")
        nc.vector.memset(cone[:], 1.0)

        # ---- phase A: load xT once, LN stats via PE ----
        xtf = [sb.tile([P, L2], BF16, tag=f"b8{k}", name=f"xt{k}") for k in range(KD)]
        xt = [t[:, 0:L] for t in xtf]
        for k in range(KD):
            (nc.sync if k % 2 == 0 else nc.scalar).dma_start(
                xt[k], dram["xT"][k * P:(k + 1) * P, :])
        pssum = ppA.tile([1, L], F32, tag="pa", name="st0")
        pssq = ppA.tile([1, L], F32, tag="pa", name="st1")
        for k in range(KD):
            xsqf = sb.tile([P, L2], BF16, tag=f"b8{4 + k % 2}", name=f"sq{k % 2}")
            xsq = xsqf[:, 0:L]
            nc.scalar.activation(xsq, xt[k], AF.Square)
            for c in range(NT):
                sl = slice(c * 512, (c + 1) * 512)
                nc.tensor.matmul(pssum[:, sl], onesf[:], xt[k][:, sl],
                                 start=(k == 0), stop=(k == KD - 1))
                nc.tensor.matmul(pssq[:, sl], onesf[:], xsq[:, sl],
                                 start=(k == 0), stop=(k == KD - 1))
        mu = sb.tile([1, L], F32, tag="s0", name="mu")
        m2 = sb.tile([1, L], F32, tag="s1", name="m2")
        nc.scalar.activation(mu[:], pssum[:], AF.Copy, scale=1.0 / DIM)
        nc.scalar.activation(m2[:], pssq[:], AF.Copy, scale=1.0 / DIM)
        mu2 = ppA.tile([1, L], F32, tag="pa", name="mu2")
        nc.vector.tensor_tensor(mu2[:], mu[:], mu[:], OP.mult)
        var = ppA.tile([1, L], F32, tag="pa", name="var")
        nc.vector.tensor_tensor(var[:], m2[:], mu2[:], OP.subtract)
        lnv = ppA.tile([1, L], F32, tag="pa", name="lnv")
        nc.scalar.activation(lnv[:], var[:], AF.Ln, bias=ceps[:])
        rstd = sb.tile([1, L], BF16, tag="s2", name="rstd")
        nc.scalar.activation(rstd[:], lnv[:], AF.Exp, scale=-0.5)
        nc.sync.dma_start(rrow[:], rstd[:])
        r1rhs = sb.tile([2, L], BF16, tag="r1r", name="r1r")
        # row0 = +mu*rstd (host negates wsum); row1 = ones (memset full, row0
        # overwritten after - engines cannot address partition offset 1)
        nc.vector.memset(r1rhs[:], 1.0)
        nc.vector.tensor_tensor(r1rhs[0:1, :], mu[:], rstd[:], OP.mult)
        rstd_b = sb.tile([P, L], BF16, tag="rb", name="rb")
        nc.sync.dma_start(rstd_b[:], rrow[0:1, :].broadcast_to([P, L]))
        xs = [sb.tile([P, L], BF16, tag=f"s4{k}", name=f"xs{k}") for k in range(KD)]
        for k in range(KD):
            nc.vector.tensor_tensor(xs[k][:], xt[k], rstd_b[:], OP.mult)

        # ---- phase B+C: in_proj (+rank-1 LN fold) and conv, interleaved ----
        ppA.release()
        ppB = tc.alloc_tile_pool(name="ppB", bufs=2, space="PSUM")
        ppC = tc.alloc_tile_pool(name="ppC", bufs=2, space="PSUM")
        ppX = tc.alloc_tile_pool(name="ppX", bufs=1, space="PSUM")   # [64,L] xproj
        u2 = [sb.tile([P, L2], BF16, tag=f"u2{h}", name=f"u2{h}") for h in range(2)]
        zs2 = [sb.tile([P, L2], BF16, tag=f"zs{h}", name=f"zs{h}") for h in range(2)]
        px = ppX.tile([64, L], F32, tag="px", name="px")

        xcs = {}
        uos = {}
        for m in range(MI):
            if m < MX:
                xcm = sb.tile([P, L + XPAD], BF16, tag="xc0", name=f"xc{m}")
                nc.vector.memset(xcm[:, 0:XPAD], 0.0)
                xcs[m] = xcm
            for c in range(NT):
                sl = slice(c * 512, (c + 1) * 512)
                pmm = ppB.tile([P, 512], F32, tag="mmb", name="mmb")
                for k in range(KD):
                    nc.tensor.matmul(pmm[:], inw[k][:, m * P:(m + 1) * P],
                                     xs[k][:, sl], start=(k == 0), stop=False)
                nc.tensor.matmul(pmm[:], inr1[:, m * P:(m + 1) * P],
                                 r1rhs[:, sl], start=False, stop=True)
                if m < MX:
                    psl = slice(XPAD + c * 512, XPAD + (c + 1) * 512)
                    if c % 2 == 0:
                        nc.scalar.activation(xcs[m][:, psl], pmm[:], AF.Copy)
                    else:
                        nc.vector.tensor_copy(xcs[m][:, psl], pmm[:])
                else:
                    h = (m - MX) // 2
                    seg = ((m - MX) % 2) * L
                    nc.scalar.activation(zs2[h][:, seg + c * 512:seg + (c + 1) * 512],
                                         pmm[:], AF.Silu)
            if m < MX:
                # depthwise conv: 4 shifted diag-matmuls; SiLU on evacuation.
                # m<4 are this core's scan channels (u2 pair tiles); m 4..7
                # only feed xproj (transient uo tiles).
                if m < 4:
                    udst, uoff = u2[m // 2], (m % 2) * L
                else:
                    uos[m] = sb.tile([P, L], BF16, tag=("rb", "uo1")[m % 2],
                                     name=f"uo{m}")
                    udst, uoff = uos[m], 0
                for c in range(NT):
                    pcv = ppC.tile([P, 512], F32, tag="cv", name="cv")
                    for k in range(DCONV):
                        wsl = slice((m * DCONV + k) * P, (m * DCONV + k + 1) * P)
                        off = XPAD - 3 + k + c * 512
                        nc.tensor.matmul(pcv[:], convd[:, wsl],
                                         xcs[m][:, off:off + 512],
                                         start=(k == 0), stop=(k == DCONV - 1))
                    nc.scalar.activation(
                        udst[:, uoff + c * 512:uoff + (c + 1) * 512],
                        pcv[:], AF.Silu, bias=convb[:, m:m + 1])
                # xproj contribution for this m (after u is ready)
                for c in range(NT):
                    sl = slice(c * 512, (c + 1) * 512)
                    nc.tensor.matmul(px[:, sl], xpw[m][:],
                                     udst[:, uoff + c * 512:uoff + (c + 1) * 512],
                                     start=(m == 0), stop=(m == MX - 1))

        # ---- phase D: dbl, dtproj, softplus, dtu ----
        dbl = sb.tile([64, L], BF16, tag="dbl", name="dbl")
        nc.scalar.activation(dbl[:], px[:], AF.Copy)
        nc.sync.dma_start(brows[:], dbl[DTRANK:DTRANK + DSTATE, :])
        nc.scalar.dma_start(crows[:], dbl[DTRANK + DSTATE:64, :])
        ppX.release()
        ppC.release()
        ppB.release()
        ppD = tc.alloc_tile_pool(name="ppD", bufs=1, space="PSUM")   # [128,L]
        dt2 = [sb.tile([P, L2], BF16, tag=f"dt{h}", name=f"dt{h}") for h in range(2)]
        for h in range(2):
            for s in range(2):
                m = h * 2 + s
                pd = ppD.tile([P, L], F32, tag="pd", name="pd")
                for c in range(NT):
                    sl = slice(c * 512, (c + 1) * 512)
                    nc.tensor.matmul(pd[:, sl], dtpw[:, m * P:(m + 1) * P],
                                     dbl[0:DTRANK, sl], start=True, stop=True)
                seg = slice(s * L, (s + 1) * L)
                nc.scalar.activation(dt2[h][:, seg], pd[:], AF.Exp,
                                     bias=dtb[:, m:m + 1])
        for h in range(2):
            nc.scalar.activation(dt2[h][:, 0:L], dt2[h][:, 0:L], AF.Ln, bias=cone[:])
            nc.scalar.activation(dt2[h][:, L:L2], dt2[h][:, L:L2], AF.Ln, bias=cone[:])
        dtu2 = [sb.tile([P, L2], BF16, tag=f"du{h}", name=f"du{h}") for h in range(2)]
        for h in range(2):
            nc.vector.tensor_tensor(dtu2[h][:], dt2[h][:], u2[h][:], OP.mult)
            # poison the pair boundary so the fused scan's carry dies there
            nc.vector.memset(dt2[h][:, L:L + 1], 30.0)

        # ---- phase E: 2 pair-passes x 16 states ----
        ppD.release()
        ppE = tc.alloc_tile_pool(name="ppE", bufs=1, space="PSUM")   # [128,L2]
        ym2 = [None, None]
        for h in range(2):
            yp = ppE.tile([P, L2], F32, tag="yp", name="yp")
            for n in range(1, DSTATE + 1):
                bn = sb.tile([P, L], BF16, tag=f"s4{n % 2}", name="bn")
                cn = sb.tile([P, L], BF16, tag=f"s4{2 + n % 2}", name="cn")
                nc.sync.dma_start(bn[:], brows[n - 1:n, :].broadcast_to([P, L]))
                nc.scalar.dma_start(cn[:], crows[n - 1:n, :].broadcast_to([P, L]))
                dA = sb.tile([P, L2], BF16, tag=f"b8{n % 2}", name="dA")
                nc.scalar.activation(dA[:, 0:L], dt2[h][:, 0:L], AF.Exp,
                                     scale=float(-n))
                nc.scalar.activation(dA[:, L:L2], dt2[h][:, L:L2], AF.Exp,
                                     scale=float(-n))
                dBu = sb.tile([P, L2], BF16, tag=f"b8{2 + n % 2}", name="dBu")
                nc.vector.tensor_tensor(dBu[:, 0:L], dtu2[h][:, 0:L], bn[:], OP.mult)
                nc.vector.tensor_tensor(dBu[:, L:L2], dtu2[h][:, L:L2], bn[:], OP.mult)
                hh = sb.tile([P, L2], BF16, tag=f"b8{4 + n % 2}", name="hh")
                nc.vector.tensor_tensor_scan(hh[:], dA[:], dBu[:], 0.0,
                                             OP.mult, OP.add)
                hc = sb.tile([P, L2], BF16, tag=f"b8{2 + n % 2}", name="hc")
                nc.vector.tensor_tensor(hc[:, 0:L], hh[:, 0:L], cn[:], OP.mult)
                nc.vector.tensor_tensor(hc[:, L:L2], hh[:, L:L2], cn[:], OP.mult)
                for i in range(2 * NT):
                    sl = slice(i * 512, (i + 1) * 512)
                    nc.tensor.matmul(yp[:, sl], ident[:], hc[:, sl],
                                     start=(n == 1), stop=False)
            # += diag(D) @ u  (D*u term), closes each slice's accumulation
            for s in range(2):
                m = h * 2 + s
                for c in range(NT):
                    sl = slice(s * L + c * 512, s * L + (c + 1) * 512)
                    nc.tensor.matmul(yp[:, sl], dvecd[:, m * P:(m + 1) * P],
                                     u2[h][:, sl], start=False, stop=True)
            yps = sb.tile([P, L2], BF16, tag=f"b8{h}", name="yps")
            nc.scalar.activation(yps[:, 0:L], yp[:, 0:L], AF.Copy)
            nc.scalar.activation(yps[:, L:L2], yp[:, L:L2], AF.Copy)
            ym2[h] = sb.tile([P, L2], BF16, tag=f"u2{h}", name=f"ym{h}")
            nc.vector.tensor_tensor(ym2[h][:], yps[:], zs2[h][:], OP.mult)

        # ---- phase F: out_proj (weff = fuse @ out_w, premultiplied) ----
        ppE.release()
        ppF = tc.alloc_tile_pool(name="ppF", bufs=2, space="PSUM")
        for half in range(2):
            po = [ppF.tile([P, L], F32, tag="po", name="po") for _ in range(2)]
            for k in range(MH):
                h, seg = k // 2, (k % 2) * L
                for j in range(2):
                    mo = half * 2 + j
                    for c in range(NT):
                        sl = slice(c * 512, (c + 1) * 512)
                        nc.tensor.matmul(po[j][:, sl],
                                         weff[k][:, mo * P:(mo + 1) * P],
                                         ym2[h][:, seg + c * 512:seg + (c + 1) * 512],
                                         start=(k == 0), stop=(k == MH - 1))
            for j in range(2):
                mo = half * 2 + j
                for c in range(NT):
                    sl = slice(c * 512, (c + 1) * 512)
                    ev = sb.tile([P, 512], F32, tag=f"ev{(j * NT + c) % 2}", name="ev")
                    if (j * NT + c) % 2 == 0:
                        nc.scalar.activation(ev[:], po[j][:, sl], AF.Copy)
                    else:
                        nc.vector.tensor_copy(ev[:], po[j][:, sl])
                    (nc.sync if c % 2 == 0 else nc.scalar).dma_start(
                        outT[mo * P:(mo + 1) * P, sl], ev[:])
        ppF.release()
    nc.compile()
    return nc


def _host_prep(inputs):
    f32 = np.float32
    x = np.asarray(inputs["x"], f32)
    ln_g = np.asarray(inputs["ln_g"], f32); ln_b = np.asarray(inputs["ln_b"], f32)
    in_w = np.asarray(inputs["in_w"], f32)
    conv_w = np.asarray(inputs["conv_w"], f32); conv_b = np.asarray(inputs["conv_b"], f32)
    xproj_w = np.asarray(inputs["xproj_w"], f32); dtproj_w = np.asarray(inputs["dtproj_w"], f32)
    dt_bias = np.asarray(inputs["dt_bias"], f32)
    D = np.asarray(inputs["D"], f32)
    out_w = np.asarray(inputs["out_w"], f32)
    fuse_w = np.asarray(inputs["fuse_w"], f32)

    maps = []
    for p in range(4):
        dir_, b = p // 2, p % 2
        W = in_w[dir_] * ln_g[None, :]          # [2*Di, D], LN gain folded
        in_bias_full = in_w[dir_] @ ln_b        # LN bias folded
        Weff_out = fuse_w[:, dir_ * DIM:(dir_ + 1) * DIM] @ out_w[dir_]
        xb = x[b] if dir_ == 0 else x[b, ::-1]
        for half in range(2):
            sl = slice(half * HALF, (half + 1) * HALF)
            # permute xc channels so this core's scan channels are rows 0..511
            perm = np.concatenate([np.arange(half * HALF, (half + 1) * HALF),
                                   np.arange((1 - half) * HALF, (2 - half) * HALF)])
            rows = np.concatenate([perm, DINNER + np.arange(half * HALF, (half + 1) * HALF)])
            Wr = W[rows]
            convd = np.zeros((P, MX * DCONV * P), f32)
            cw = conv_w[dir_][perm]             # [DINNER, DCONV]
            for j in range(MX):
                for k in range(DCONV):
                    blk = (j * DCONV + k) * P
                    convd[:, blk:blk + P] = np.diag(cw[j * P:(j + 1) * P, k])
            dvecd = np.zeros((P, MH * P), f32)
            dv = D[dir_, sl]
            for mm in range(MH):
                dvecd[:, mm * P:(mm + 1) * P] = np.diag(dv[mm * P:(mm + 1) * P])
            m = dict(
                xT=np.ascontiguousarray(xb.T.astype(BF)),
                inw=np.ascontiguousarray(Wr.T.astype(BF)),
                inr1=np.ascontiguousarray(
                    np.stack([-Wr.sum(1), in_bias_full[rows]]).astype(BF)),
                convd=convd.astype(BF),
                convb=np.ascontiguousarray(conv_b[dir_][perm].reshape(MX, P).T),
                xpw=np.ascontiguousarray(xproj_w[dir_][:, perm].T.astype(BF)),
                dtpw=np.ascontiguousarray(dtproj_w[dir_, sl].T.astype(BF)),
                dtb=np.ascontiguousarray(dt_bias[dir_, sl].reshape(MH, P).T),
                dvecd=dvecd.astype(BF),
                weff=np.ascontiguousarray(Weff_out[:, sl].T.astype(BF)),
                onesf=np.ones((P, 1), np.float32).astype(BF),
                ident=np.eye(P, dtype=np.float32).astype(BF),
            )
            maps.append(m)
    return maps


def kernel(**inputs):
    if "nc" not in _CACHE:
        _CACHE["nc"] = _build()
    nc = _CACHE["nc"]
    maps = _host_prep(inputs)
    res = run_bass_kernel_spmd(nc, maps, list(range(8)))
    x = np.asarray(inputs["x"], np.float32)
    fuse_b = np.asarray(inputs["fuse_b"], np.float32)
    out = x + fuse_b[None, None, :]
    for p in range(4):
        dir_, b = p // 2, p % 2
        for half in range(2):
            pt = np.asarray(res.results[p * 2 + half]["outT"], np.float32).T
            if dir_ == 1:
                pt = pt[::-1]
            out[b] += pt
    return out.astype(np.float32)
